# revision 21
# baseline (speedup 1.0000x reference)
"""AttentionPool TRN2 kernel.

Problem: B=2048, S=512, D=128, H=4, T=8 (Q = T*H = 32), C=64.
  k = keys @ Wk^T ; v = keys @ Wv^T
  q = q_flat + (ctx @ Wc^T + bc).reshape(B, Q, D)
  attn = (q @ k^T) * scale * inv_t[q] - slopes[q] * games_ago[s]
  out  = softmax_masked(attn) @ v            -> [B, T, H*D]

Restructured so `keys` is touched by exactly two matmuls per row:
  logits[q,s] = qk'[q,:]  . keys[s,:]        (qk' = (q @ Wk) * scale*inv_t, host-folded)
  pooled[q,:] = (w[q,:] @ keys) @ Wv^T
The mask matmul (stationary mstat [8,128], rhs maskp [8,512]) folds in:
  rows 0-3: -MASK_NEG on masked positions (host: MASK_NEG*(mask-1))
  rows 4-7: -SC*slope[p]*games_ago = SC*slope[p]*(s - n_r + 1)
so lg_ps/SC is exactly the true masked logit (softmax shift-invariance,
|true logit| <~ 0.5 at these weight scales) and exp needs no row max.

keys are cast f32->f16 in-flight by the SWDGE DMA (gpsimd), so no
engine spends element time on the conversion.

Sharding: pure data parallel over batch, 256 rows/core on 8 cores.
"""

import sys

if "/opt/trn_rl_repo" not in sys.path:
    sys.path.insert(0, "/opt/trn_rl_repo")

import numpy as np

import concourse.bacc as bacc
import concourse.bass as bass
import concourse.tile as tile
from concourse import mybir
from concourse.bass_utils import run_bass_kernel_spmd

B, S, D, H, T, C = 2048, 512, 128, 4, 8, 64
Q = T * H  # 32
N_CORES = 8
ROWS = B // N_CORES  # 256 rows per core
GRP = 4  # batch rows per group -> 4*32 = 128 partitions
BLK = 128  # rows per block (ctx/QKT staging)
SC = 64.0  # power-of-two prescale keeping fp16 operands in normal range
MASK_NEG = 16384.0  # fp16/f32-exact; /SC = 256 pushes masked logits below -126

F32 = mybir.dt.float32
F16 = mybir.dt.float16

N_GRP = ROWS // GRP  # 64


def _emit(nc, tc, rows):
    """Emit the per-core program for `rows` batch rows (rows % GRP == 0)."""
    n_grp_total = rows // GRP
    keys_d = nc.declare_dram_parameter("keys", [rows, S, D], F32, isOutput=False)
    maskp_d = nc.declare_dram_parameter("maskp", [8, n_grp_total, S], F16,
                                        isOutput=False)
    ctx_d = nc.declare_dram_parameter("ctx", [rows, C], F32, isOutput=False)
    maug_d = nc.declare_dram_parameter("maug", [C + 1, Q, D], F16, isOutput=False)
    wvt_d = nc.declare_dram_parameter("wvt", [D, D], F16, isOutput=False)
    mstat_d = nc.declare_dram_parameter("mstat", [8, 128], F16, isOutput=False)
    id16_d = nc.declare_dram_parameter("id16", [128, 128], F16, isOutput=False)
    id32_d = nc.declare_dram_parameter("id32", [128, 128], F32, isOutput=False)
    out_d = nc.declare_dram_parameter("out", [rows, Q * D], F16, isOutput=True)

    keys_ap = keys_d.ap()
    ctx_ap = ctx_d.ap()
    out_ap = out_d.ap()

    n_blk = (rows + BLK - 1) // BLK

    import contextlib

    with contextlib.ExitStack() as ctx:
        singles = ctx.enter_context(tc.tile_pool(name="singles", bufs=1))
        kpool = ctx.enter_context(tc.tile_pool(name="kpool", bufs=4))
        ktpool = ctx.enter_context(tc.tile_pool(name="ktpool", bufs=4))
        blkpool = ctx.enter_context(tc.tile_pool(name="blkpool", bufs=2))
        qktpool = ctx.enter_context(tc.tile_pool(name="qktpool", bufs=2))
        work = ctx.enter_context(tc.tile_pool(name="work", bufs=4))
        small = ctx.enter_context(tc.tile_pool(name="small", bufs=6))
        ps = ctx.enter_context(tc.tile_pool(name="ps", bufs=1, space="PSUM"))

        # ---- constants (loaded once; ids first — they gate the first
        # transposes — big maskp last) ----
        id32_sb = singles.tile([128, 128], F32)
        nc.sync.dma_start(out=id32_sb, in_=id32_d.ap())
        id16_sb = singles.tile([128, 128], F16)
        nc.sync.dma_start(out=id16_sb, in_=id16_d.ap())
        maug_sb = singles.tile([C + 1, Q, D], F16)
        nc.sync.dma_start(out=maug_sb, in_=maug_d.ap())
        wvt_sb = singles.tile([D, D], F16)
        nc.sync.dma_start(out=wvt_sb, in_=wvt_d.ap())
        mstat_sb = singles.tile([8, 128], F16)
        nc.sync.dma_start(out=mstat_sb, in_=mstat_d.ap())
        maskp_sb = singles.tile([8, n_grp_total, S], F16)
        nc.sync.dma_start(out=maskp_sb, in_=maskp_d.ap())

        # Keys land s-permuted: partition p holds s = 4p+j (j = 0..3), so each
        # partition reads one contiguous 2 KiB run per row (line-rate DMA).
        # Pass 1/2 use the same s<->(p, j) mapping; maskp is host-permuted.
        PAIR = 4 * GRP
        NPG = PAIR // GRP  # groups per load
        staged = {}

        def _load_pair(gp, split=False):
            if NPG * gp >= n_grp_total or NPG * gp in staged:
                return
            g0 = NPG * gp * GRP
            pn = min(PAIR, rows - g0)
            k16 = kpool.tile([128, PAIR, S // 128, D], F16, tag="k16",
                             name=f"k16_{gp}")
            # SWDGE cast-DMA: HBM f32 -> SBUF fp16 in one transfer
            if split:
                for h in range(0, pn, GRP):
                    nc.gpsimd.dma_start(
                        out=k16[:, h : h + GRP],
                        in_=keys_ap[g0 + h : g0 + h + GRP].rearrange(
                            "r (p j) d -> p r j d", p=128),
                    )
            else:
                nc.gpsimd.dma_start(
                    out=k16[:, :pn],
                    in_=keys_ap[g0 : g0 + pn].rearrange("r (p j) d -> p r j d", p=128),
                )
            for h in range(pn // GRP):
                staged[NPG * gp + h] = k16[:, h * GRP : (h + 1) * GRP]

        PF = 3  # software prefetch distance (pairs)
        for gp in range(PF):
            _load_pair(gp, split=(gp == 0))

        # ---- prologue: conditioned queries qk'^T for every block ----
        qkt_blocks = []
        for blk in range(n_blk):
            r0 = blk * BLK
            bn = min(BLK, rows - r0)
            assert bn % GRP == 0

            ctx_sb = blkpool.tile([BLK, C], F32, tag="ctx")
            nc.sync.dma_start(out=ctx_sb[:bn], in_=ctx_ap[r0 : r0 + bn])
            ctxt_ps = ps.tile([C, BLK], F32, tag="t32", bufs=3)
            nc.tensor.transpose(ctxt_ps[:, :bn], ctx_sb[:bn], id32_sb[:bn, :bn])
            ctxt_sb = blkpool.tile([C + 1, BLK], F16, tag="ctxt")
            nc.vector.tensor_copy(out=ctxt_sb[:C, :bn], in_=ctxt_ps[:, :bn])
            nc.vector.memset(ctxt_sb[C : C + 1, :bn], 1.0)

            # qk'^T for the block: [D, bn, Q] fp16 (prescaled by SC*scale*inv_t)
            qkt_sb = qktpool.tile([D, BLK, Q], F16, tag="qkt")
            for q in range(Q):
                qkt_ps = ps.tile([D, BLK], F32, tag="t32", bufs=3)
                nc.tensor.matmul(
                    qkt_ps[:, :bn], maug_sb[:, q, :], ctxt_sb[:, :bn],
                    start=True, stop=True,
                )
                nc.vector.tensor_copy(out=qkt_sb[:, :bn, q], in_=qkt_ps[:, :bn])
            qkt_blocks.append(qkt_sb)

        for g in range(n_grp_total):
            g0 = g * GRP  # absolute row of this group
            qkt_sb = qkt_blocks[g0 // BLK]
            gl = (g0 % BLK) // GRP
            if g % NPG == 0:
                _load_pair(g // NPG + PF)
            k16 = staged.pop(g)
            maskp_g = maskp_sb[:, g, :]

            # ---- keys^T per row: [d, s] via transpose-mode (fp16 PSUM out
            # halves the PSUM->SBUF copy cost; copies split Scalar/Vector) ----
            kt_sb = ktpool.tile([128, GRP, S], F16, tag="kt")
            for r in range(GRP):
                ktp = ps.tile([128, S], F16, tag="t16", bufs=3)
                for c in range(S // 128):
                    nc.tensor.transpose(
                        ktp[:, c * 128 : (c + 1) * 128], k16[:, r, c, :], id16_sb
                    )
                if r % 2 == 0:
                    nc.scalar.copy(out=kt_sb[:, r, :], in_=ktp)
                else:
                    nc.vector.tensor_copy(out=kt_sb[:, r, :], in_=ktp)

            # ---- pass 1: logits = qk'.keys + mask/alibi matmul ----
            lg_ps = ps.tile([128, S], F32, tag="logits", bufs=2)
            for r in range(GRP):
                nc.tensor.matmul(
                    lg_ps[32 * r : 32 * (r + 1), :],
                    qkt_sb[:, gl * GRP + r, :],
                    kt_sb[:, r, :],
                    start=True, stop=False,
                    tile_position=(0, 32 * r),
                    skip_group_check=True,
                )
            nc.tensor.matmul(
                lg_ps, mstat_sb, maskp_g,
                start=False, stop=True,
                skip_group_check=True,
            )

            # lg/SC is the exact shifted logit; exp directly from PSUM,
            # fp16 out, accumulator gives the softmax denominator.
            e_sb = work.tile([128, S], F16, tag="e")
            sum_sb = small.tile([128, 1], F32, tag="sum")
            nc.scalar.activation(
                out=e_sb, in_=lg_ps, func=mybir.ActivationFunctionType.Exp,
                scale=1.0 / SC, accum_out=sum_sb,
            )
            rs_sb = small.tile([128, 1], F32, tag="rs")
            nc.vector.reciprocal(rs_sb, sum_sb)

            # ---- w^T: [s_in_chunk, c, rq] fp16 via matmul-against-identity ----
            wt_ps = ps.tile([128, S // 128, 128], F32, tag="t32", bufs=3)
            for c in range(S // 128):
                nc.tensor.matmul(
                    wt_ps[:, c, :], e_sb[:, c * 128 : (c + 1) * 128],
                    id16_sb, start=True, stop=True,
                )
            wt_sb = work.tile([128, S // 128, 128], F16, tag="wt")
            nc.vector.tensor_copy(out=wt_sb, in_=wt_ps)

            # ---- pass 2: pk^T[d, rq] = sum_s keys[s,d] * w[rq,s] ----
            pk_ps = ps.tile([128, 128], F32, tag="t32", bufs=3)
            for r in range(GRP):
                for c in range(S // 128):
                    nc.tensor.matmul(
                        pk_ps[:, 32 * r : 32 * (r + 1)],
                        k16[:, r, c, :],
                        wt_sb[:, c, 32 * r : 32 * (r + 1)],
                        start=(c == 0), stop=(c == S // 128 - 1),
                        skip_group_check=True,
                    )
            pkt_sb = work.tile([128, 128], F16, tag="pkt")
            nc.vector.tensor_copy(out=pkt_sb, in_=pk_ps)

            # ---- pooled[rq, e] = pk^T.T @ Wv^T  (fp16 in, fp32 accum) ----
            po_ps = ps.tile([128, 128], F32, tag="t32", bufs=3)
            nc.tensor.matmul(po_ps, pkt_sb, wvt_sb, start=True, stop=True)

            o_sb = work.tile([128, 128], F16, tag="o")
            nc.vector.tensor_scalar(
                out=o_sb, in0=po_ps, scalar1=rs_sb, scalar2=None,
                op0=mybir.AluOpType.mult,
            )
            nc.sync.dma_start(
                out=out_ap[g0 : g0 + GRP].rearrange("r (q e) -> (r q) e", e=D),
                in_=o_sb,
            )


def _build(rows):
    nc = bacc.Bacc(trn_type="TRN2", target_bir_lowering=False, debug=False)
    with tile.TileContext(nc) as tc:
        _emit(nc, tc, rows)
    nc.compile()
    return nc


def host_consts(queries, Wk, log_temperature, Wc, bc, Wv):
    """Fold projections/scales into small host-side constants."""
    queries = np.asarray(queries, np.float64)
    Wk = np.asarray(Wk, np.float64)
    Wc = np.asarray(Wc, np.float64)
    bc = np.asarray(bc, np.float64)
    Wv = np.asarray(Wv, np.float64)
    lt = np.asarray(log_temperature, np.float64)

    scale = D ** -0.5
    inv_t = np.repeat(np.exp(-lt), H)  # [Q]
    slopes_h = 2.0 ** (-8.0 * (np.arange(H) + 1) / H)
    slopes = np.tile(slopes_h, T)  # [Q]
    s_q = scale * inv_t  # [Q]

    q_eff = queries.reshape(Q, D) + bc.reshape(Q, D)  # [Q, D]
    qk0 = q_eff @ Wk  # [Q, D]
    # maug[c, q, d]: rows 0..C-1 = SC*s_q * (Wc_q^T @ Wk); row C = SC*s_q * qk0
    maug = np.empty((C + 1, Q, D), np.float64)
    for q in range(Q):
        Wc_q = Wc[q * D : (q + 1) * D, :]  # [D(e), C]
        maug[:C, q, :] = (Wc_q.T @ Wk) * (SC * s_q[q])
        maug[C, q, :] = qk0[q] * (SC * s_q[q])

    # mstat: stationary for the mask/alibi matmul, k=8 partitions.
    # rows 0-3: select row r's 32 partitions (mask term)
    # rows 4-7: SC*slope[p] on row r's partitions (alibi term)
    slope_col = np.tile(slopes, 128 // Q)  # [128], p -> slopes[p % 32]
    mstat = np.zeros((8, 128), np.float16)
    for r in range(GRP):
        mstat[r, 32 * r : 32 * (r + 1)] = 1.0
        mstat[4 + r, 32 * r : 32 * (r + 1)] = SC * slope_col[32 * r : 32 * (r + 1)]

    return dict(
        maug=maug.astype(np.float16),
        wvt=np.ascontiguousarray(Wv.T).astype(np.float16),
        mstat=mstat,
        id16=np.eye(128, dtype=np.float16),
        id32=np.eye(128, dtype=np.float32),
    )


def make_in_maps(keys, mask, context, consts, rows, n_cores):
    keys = np.asarray(keys, np.float32)
    mask01 = np.asarray(mask, bool)
    ctx = np.asarray(context, np.float32)
    n_grp = rows // GRP
    s_ar = np.arange(S, dtype=np.float32)[None, None, :]  # [1,1,S]
    in_maps = []
    for i in range(n_cores):
        sl = slice(i * rows, (i + 1) * rows)
        m = mask01[sl].reshape(n_grp, GRP, S)  # [g, r, s]
        n_real = m.sum(axis=2, dtype=np.float32)  # [g, r]
        maskp = np.empty((8, n_grp, S), np.float32)
        # rows 0-3: 0 on real positions, -MASK_NEG on masked
        maskp[0:4] = (m.transpose(1, 0, 2).astype(np.float32) - 1.0) * MASK_NEG
        # rows 4-7: s - n_r + 1  (== -games_ago on real positions)
        maskp[4:8] = s_ar - n_real.T[:, :, None] + 1.0
        # device kt/lg column c' = j*128 + p holds s = 4p + j
        perm = 4 * (np.arange(S) % 128) + np.arange(S) // 128
        maskp = maskp[:, :, perm]
        in_maps.append(
            dict(
                keys=np.ascontiguousarray(keys[sl]),
                maskp=maskp.astype(np.float16),
                ctx=np.ascontiguousarray(ctx[sl]),
                **consts,
            )
        )
    return in_maps


_cache = {}


def run(keys, mask, context, queries, Wk, Wv, log_temperature, Wc, bc,
        trace=False, **kw):
    consts = host_consts(queries, Wk, log_temperature, Wc, bc, Wv)
    if ROWS not in _cache:
        _cache[ROWS] = _build(ROWS)
    nc = _cache[ROWS]
    in_maps = make_in_maps(keys, mask, context, consts, ROWS, N_CORES)
    res = run_bass_kernel_spmd(nc, in_maps, core_ids=list(range(N_CORES)),
                               trace=trace, **kw)
    out = np.concatenate([res.results[i]["out"] for i in range(N_CORES)], axis=0)
    return out.reshape(B, T, H * D).astype(np.float32), res


def kernel(keys, mask, context, queries, Wk, Wv, log_temperature, Wc, bc):
    out, _ = run(keys, mask, context, queries, Wk, Wv, log_temperature, Wc, bc)
    return out


# revision 22
# speedup vs baseline: 1.1264x; 1.1264x over previous
"""AttentionPool TRN2 kernel.

Problem: B=2048, S=512, D=128, H=4, T=8 (Q = T*H = 32), C=64.
  k = keys @ Wk^T ; v = keys @ Wv^T
  q = q_flat + (ctx @ Wc^T + bc).reshape(B, Q, D)
  attn = (q @ k^T) * scale * inv_t[q] - slopes[q] * games_ago[s]
  out  = softmax_masked(attn) @ v            -> [B, T, H*D]

Restructured so `keys` is touched by exactly two matmuls per row:
  logits[q,s] = qk'[q,:]  . keys[s,:]        (qk' = (q @ Wk) * scale*inv_t, host-folded)
  pooled[q,:] = (w[q,:] @ keys) @ Wv^T
The mask matmul (stationary mstat [8,128], rhs maskp [8,512]) folds in:
  rows 0-3: -MASK_NEG on masked positions (host: MASK_NEG*(mask-1))
  rows 4-7: -SC*slope[p]*games_ago = SC*slope[p]*(s - n_r + 1)
so lg_ps/SC is exactly the true masked logit (softmax shift-invariance,
|true logit| <~ 0.5 at these weight scales) and exp needs no row max.

keys are cast f32->f16 in-flight by the SWDGE DMA (gpsimd), so no
engine spends element time on the conversion.

Sharding: pure data parallel over batch, 256 rows/core on 8 cores.
"""

import sys

if "/opt/trn_rl_repo" not in sys.path:
    sys.path.insert(0, "/opt/trn_rl_repo")

import numpy as np

import concourse.bacc as bacc
import concourse.bass as bass
import concourse.tile as tile
from concourse import mybir
from concourse.bass_utils import run_bass_kernel_spmd

B, S, D, H, T, C = 2048, 512, 128, 4, 8, 64
Q = T * H  # 32
N_CORES = 8
ROWS = B // N_CORES  # 256 rows per core
GRP = 4  # batch rows per group -> 4*32 = 128 partitions
BLK = 128  # rows per block (ctx/QKT staging)
SC = 64.0  # power-of-two prescale keeping fp16 operands in normal range
MASK_NEG = 16384.0  # fp16/f32-exact; /SC = 256 pushes masked logits below -126

F32 = mybir.dt.float32
F16 = mybir.dt.float16

N_GRP = ROWS // GRP  # 64


def _emit(nc, tc, rows):
    """Emit the per-core program for `rows` batch rows (rows % GRP == 0)."""
    n_grp_total = rows // GRP
    keys_d = nc.declare_dram_parameter("keys", [rows, S, D], F32, isOutput=False)
    maskp_d = nc.declare_dram_parameter("maskp", [8, n_grp_total, S], F16,
                                        isOutput=False)
    ctx_d = nc.declare_dram_parameter("ctx", [rows, C], F32, isOutput=False)
    maug_d = nc.declare_dram_parameter("maug", [C + 1, Q, D], F16, isOutput=False)
    wvt_d = nc.declare_dram_parameter("wvt", [D, D], F16, isOutput=False)
    mstat_d = nc.declare_dram_parameter("mstat", [8, 128], F16, isOutput=False)
    id16_d = nc.declare_dram_parameter("id16", [128, 128], F16, isOutput=False)
    id32_d = nc.declare_dram_parameter("id32", [128, 128], F32, isOutput=False)
    out_d = nc.declare_dram_parameter("out", [rows, Q * D], F32, isOutput=True)

    keys_ap = keys_d.ap()
    ctx_ap = ctx_d.ap()
    out_ap = out_d.ap()

    n_blk = (rows + BLK - 1) // BLK

    import contextlib

    with contextlib.ExitStack() as ctx:
        singles = ctx.enter_context(tc.tile_pool(name="singles", bufs=1))
        kpool = ctx.enter_context(tc.tile_pool(name="kpool", bufs=3))
        ktpool = ctx.enter_context(tc.tile_pool(name="ktpool", bufs=4))
        blkpool = ctx.enter_context(tc.tile_pool(name="blkpool", bufs=2))
        qktpool = ctx.enter_context(tc.tile_pool(name="qktpool", bufs=2))
        work = ctx.enter_context(tc.tile_pool(name="work", bufs=4))
        small = ctx.enter_context(tc.tile_pool(name="small", bufs=6))
        ps = ctx.enter_context(tc.tile_pool(name="ps", bufs=1, space="PSUM"))

        # ---- constants (loaded once; ids first — they gate the first
        # transposes — big maskp last) ----
        id32_sb = singles.tile([128, 128], F32)
        nc.sync.dma_start(out=id32_sb, in_=id32_d.ap())
        id16_sb = singles.tile([128, 128], F16)
        nc.sync.dma_start(out=id16_sb, in_=id16_d.ap())
        maug_sb = singles.tile([C + 1, Q, D], F16)
        nc.sync.dma_start(out=maug_sb, in_=maug_d.ap())
        wvt_sb = singles.tile([D, D], F16)
        nc.sync.dma_start(out=wvt_sb, in_=wvt_d.ap())
        mstat_sb = singles.tile([8, 128], F16)
        nc.sync.dma_start(out=mstat_sb, in_=mstat_d.ap())
        maskp_sb = singles.tile([8, n_grp_total, S], F16)
        nc.sync.dma_start(out=maskp_sb, in_=maskp_d.ap())

        # Keys land s-permuted: partition p holds s = 4p+j (j = 0..3), so each
        # partition reads one contiguous 2 KiB run per row (line-rate DMA).
        # Pass 1/2 use the same s<->(p, j) mapping; maskp is host-permuted.
        PAIR = 4 * GRP
        NPG = PAIR // GRP  # groups per load
        staged = {}

        def _load_pair(gp, split=False):
            if NPG * gp >= n_grp_total or NPG * gp in staged:
                return
            g0 = NPG * gp * GRP
            pn = min(PAIR, rows - g0)
            k16 = kpool.tile([128, PAIR, S // 128, D], F16, tag="k16",
                             name=f"k16_{gp}")
            # SWDGE cast-DMA: HBM f32 -> SBUF fp16 in one transfer
            if split:
                for h in range(0, pn, GRP):
                    nc.gpsimd.dma_start(
                        out=k16[:, h : h + GRP],
                        in_=keys_ap[g0 + h : g0 + h + GRP].rearrange(
                            "r (p j) d -> p r j d", p=128),
                    )
            else:
                nc.gpsimd.dma_start(
                    out=k16[:, :pn],
                    in_=keys_ap[g0 : g0 + pn].rearrange("r (p j) d -> p r j d", p=128),
                )
            for h in range(pn // GRP):
                staged[NPG * gp + h] = k16[:, h * GRP : (h + 1) * GRP]

        PF = 2  # software prefetch distance (pairs)
        for gp in range(PF):
            _load_pair(gp, split=(gp == 0))

        # ---- prologue: conditioned queries qk'^T for every block ----
        qkt_blocks = []
        for blk in range(n_blk):
            r0 = blk * BLK
            bn = min(BLK, rows - r0)
            assert bn % GRP == 0

            ctx_sb = blkpool.tile([BLK, C], F32, tag="ctx")
            nc.sync.dma_start(out=ctx_sb[:bn], in_=ctx_ap[r0 : r0 + bn])
            ctxt_ps = ps.tile([C, BLK], F32, tag="t32", bufs=3)
            nc.tensor.transpose(ctxt_ps[:, :bn], ctx_sb[:bn], id32_sb[:bn, :bn])
            ctxt_sb = blkpool.tile([C + 1, BLK], F16, tag="ctxt")
            nc.vector.tensor_copy(out=ctxt_sb[:C, :bn], in_=ctxt_ps[:, :bn])
            nc.vector.memset(ctxt_sb[C : C + 1, :bn], 1.0)

            # qk'^T for the block: [D, bn, Q] fp16 (prescaled by SC*scale*inv_t)
            qkt_sb = qktpool.tile([D, BLK, Q], F16, tag="qkt")
            for q in range(Q):
                qkt_ps = ps.tile([D, BLK], F32, tag="t32", bufs=3)
                nc.tensor.matmul(
                    qkt_ps[:, :bn], maug_sb[:, q, :], ctxt_sb[:, :bn],
                    start=True, stop=True,
                )
                nc.vector.tensor_copy(out=qkt_sb[:, :bn, q], in_=qkt_ps[:, :bn])
            qkt_blocks.append(qkt_sb)

        for g in range(n_grp_total):
            g0 = g * GRP  # absolute row of this group
            qkt_sb = qkt_blocks[g0 // BLK]
            gl = (g0 % BLK) // GRP
            if g % NPG == 0:
                _load_pair(g // NPG + PF)
            k16 = staged.pop(g)
            maskp_g = maskp_sb[:, g, :]

            # ---- keys^T per row: [d, s] via transpose-mode (fp16 PSUM out
            # halves the PSUM->SBUF copy cost; copies split Scalar/Vector) ----
            kt_sb = ktpool.tile([128, GRP, S], F16, tag="kt")
            for r in range(GRP):
                ktp = ps.tile([128, S], F16, tag="t16", bufs=3)
                for c in range(S // 128):
                    nc.tensor.transpose(
                        ktp[:, c * 128 : (c + 1) * 128], k16[:, r, c, :], id16_sb
                    )
                if r % 2 == 0:
                    nc.scalar.copy(out=kt_sb[:, r, :], in_=ktp)
                else:
                    nc.vector.tensor_copy(out=kt_sb[:, r, :], in_=ktp)

            # ---- pass 1: logits = qk'.keys + mask/alibi matmul ----
            lg_ps = ps.tile([128, S], F32, tag="logits", bufs=2)
            for r in range(GRP):
                nc.tensor.matmul(
                    lg_ps[32 * r : 32 * (r + 1), :],
                    qkt_sb[:, gl * GRP + r, :],
                    kt_sb[:, r, :],
                    start=True, stop=False,
                    tile_position=(0, 32 * r),
                    skip_group_check=True,
                )
            nc.tensor.matmul(
                lg_ps, mstat_sb, maskp_g,
                start=False, stop=True,
                skip_group_check=True,
            )

            # lg/SC is the exact shifted logit; exp directly from PSUM,
            # fp16 out, accumulator gives the softmax denominator.
            e_sb = work.tile([128, S], F16, tag="e")
            sum_sb = small.tile([128, 1], F32, tag="sum")
            nc.scalar.activation(
                out=e_sb, in_=lg_ps, func=mybir.ActivationFunctionType.Exp,
                scale=1.0 / SC, accum_out=sum_sb,
            )
            rs_sb = small.tile([128, 1], F32, tag="rs")
            nc.vector.reciprocal(rs_sb, sum_sb)

            # ---- w^T: [s_in_chunk, c, rq] fp16 via matmul-against-identity ----
            wt_ps = ps.tile([128, S // 128, 128], F32, tag="t32", bufs=3)
            for c in range(S // 128):
                nc.tensor.matmul(
                    wt_ps[:, c, :], e_sb[:, c * 128 : (c + 1) * 128],
                    id16_sb, start=True, stop=True,
                )
            wt_sb = work.tile([128, S // 128, 128], F16, tag="wt")
            nc.vector.tensor_copy(out=wt_sb, in_=wt_ps)

            # ---- pass 2: pk^T[d, rq] = sum_s keys[s,d] * w[rq,s] ----
            pk_ps = ps.tile([128, 128], F32, tag="t32", bufs=3)
            for r in range(GRP):
                for c in range(S // 128):
                    nc.tensor.matmul(
                        pk_ps[:, 32 * r : 32 * (r + 1)],
                        k16[:, r, c, :],
                        wt_sb[:, c, 32 * r : 32 * (r + 1)],
                        start=(c == 0), stop=(c == S // 128 - 1),
                        skip_group_check=True,
                    )
            pkt_sb = work.tile([128, 128], F16, tag="pkt")
            nc.vector.tensor_copy(out=pkt_sb, in_=pk_ps)

            # ---- pooled[rq, e] = pk^T.T @ Wv^T  (fp16 in, fp32 accum) ----
            po_ps = ps.tile([128, 128], F32, tag="t32", bufs=3)
            nc.tensor.matmul(po_ps, pkt_sb, wvt_sb, start=True, stop=True)

            o_sb = work.tile([128, 128], F32, tag="o")
            nc.vector.tensor_scalar(
                out=o_sb, in0=po_ps, scalar1=rs_sb, scalar2=None,
                op0=mybir.AluOpType.mult,
            )
            nc.sync.dma_start(
                out=out_ap[g0 : g0 + GRP].rearrange("r (q e) -> (r q) e", e=D),
                in_=o_sb,
            )


def _build(rows):
    nc = bacc.Bacc(trn_type="TRN2", target_bir_lowering=False, debug=False)
    with tile.TileContext(nc) as tc:
        _emit(nc, tc, rows)
    nc.compile()
    return nc


def host_consts(queries, Wk, log_temperature, Wc, bc, Wv):
    """Fold projections/scales into small host-side constants."""
    queries = np.asarray(queries, np.float64)
    Wk = np.asarray(Wk, np.float64)
    Wc = np.asarray(Wc, np.float64)
    bc = np.asarray(bc, np.float64)
    Wv = np.asarray(Wv, np.float64)
    lt = np.asarray(log_temperature, np.float64)

    scale = D ** -0.5
    inv_t = np.repeat(np.exp(-lt), H)  # [Q]
    slopes_h = 2.0 ** (-8.0 * (np.arange(H) + 1) / H)
    slopes = np.tile(slopes_h, T)  # [Q]
    s_q = scale * inv_t  # [Q]

    q_eff = queries.reshape(Q, D) + bc.reshape(Q, D)  # [Q, D]
    qk0 = q_eff @ Wk  # [Q, D]
    # maug[c, q, d]: rows 0..C-1 = SC*s_q * (Wc_q^T @ Wk); row C = SC*s_q * qk0
    maug = np.empty((C + 1, Q, D), np.float64)
    for q in range(Q):
        Wc_q = Wc[q * D : (q + 1) * D, :]  # [D(e), C]
        maug[:C, q, :] = (Wc_q.T @ Wk) * (SC * s_q[q])
        maug[C, q, :] = qk0[q] * (SC * s_q[q])

    # mstat: stationary for the mask/alibi matmul, k=8 partitions.
    # rows 0-3: select row r's 32 partitions (mask term)
    # rows 4-7: SC*slope[p] on row r's partitions (alibi term)
    slope_col = np.tile(slopes, 128 // Q)  # [128], p -> slopes[p % 32]
    mstat = np.zeros((8, 128), np.float16)
    for r in range(GRP):
        mstat[r, 32 * r : 32 * (r + 1)] = 1.0
        mstat[4 + r, 32 * r : 32 * (r + 1)] = SC * slope_col[32 * r : 32 * (r + 1)]

    return dict(
        maug=maug.astype(np.float16),
        wvt=np.ascontiguousarray(Wv.T).astype(np.float16),
        mstat=mstat,
        id16=np.eye(128, dtype=np.float16),
        id32=np.eye(128, dtype=np.float32),
    )


def make_in_maps(keys, mask, context, consts, rows, n_cores):
    keys = np.asarray(keys, np.float32)
    mask01 = np.asarray(mask, bool)
    ctx = np.asarray(context, np.float32)
    n_grp = rows // GRP
    s_ar = np.arange(S, dtype=np.float32)[None, None, :]  # [1,1,S]
    in_maps = []
    for i in range(n_cores):
        sl = slice(i * rows, (i + 1) * rows)
        m = mask01[sl].reshape(n_grp, GRP, S)  # [g, r, s]
        n_real = m.sum(axis=2, dtype=np.float32)  # [g, r]
        maskp = np.empty((8, n_grp, S), np.float32)
        # rows 0-3: 0 on real positions, -MASK_NEG on masked
        maskp[0:4] = (m.transpose(1, 0, 2).astype(np.float32) - 1.0) * MASK_NEG
        # rows 4-7: s - n_r + 1  (== -games_ago on real positions)
        maskp[4:8] = s_ar - n_real.T[:, :, None] + 1.0
        # device kt/lg column c' = j*128 + p holds s = 4p + j
        perm = 4 * (np.arange(S) % 128) + np.arange(S) // 128
        maskp = maskp[:, :, perm]
        in_maps.append(
            dict(
                keys=np.ascontiguousarray(keys[sl]),
                maskp=maskp.astype(np.float16),
                ctx=np.ascontiguousarray(ctx[sl]),
                **consts,
            )
        )
    return in_maps


_cache = {}


def run(keys, mask, context, queries, Wk, Wv, log_temperature, Wc, bc,
        trace=False, **kw):
    consts = host_consts(queries, Wk, log_temperature, Wc, bc, Wv)
    if ROWS not in _cache:
        _cache[ROWS] = _build(ROWS)
    nc = _cache[ROWS]
    in_maps = make_in_maps(keys, mask, context, consts, ROWS, N_CORES)
    res = run_bass_kernel_spmd(nc, in_maps, core_ids=list(range(N_CORES)),
                               trace=trace, **kw)
    out = np.concatenate([res.results[i]["out"] for i in range(N_CORES)], axis=0)
    return out.reshape(B, T, H * D).astype(np.float32), res


def kernel(keys, mask, context, queries, Wk, Wv, log_temperature, Wc, bc):
    out, _ = run(keys, mask, context, queries, Wk, Wv, log_temperature, Wc, bc)
    return out


# revision 23
# speedup vs baseline: 1.2070x; 1.0716x over previous
"""AttentionPool TRN2 kernel.

Problem: B=2048, S=512, D=128, H=4, T=8 (Q = T*H = 32), C=64.
  k = keys @ Wk^T ; v = keys @ Wv^T
  q = q_flat + (ctx @ Wc^T + bc).reshape(B, Q, D)
  attn = (q @ k^T) * scale * inv_t[q] - slopes[q] * games_ago[s]
  out  = softmax_masked(attn) @ v            -> [B, T, H*D]

Restructured so `keys` is touched by exactly two matmuls per row:
  logits[q,s] = qk'[q,:]  . keys[s,:]        (qk' = (q @ Wk) * scale*inv_t, host-folded)
  pooled[q,:] = (w[q,:] @ keys) @ Wv^T
The mask matmul (stationary mstat [8,128], rhs maskp [8,512]) folds in:
  rows 0-3: -MASK_NEG on masked positions (host: MASK_NEG*(mask-1))
  rows 4-7: -SC*slope[p]*games_ago = SC*slope[p]*(s - n_r + 1)
so lg_ps/SC is exactly the true masked logit (softmax shift-invariance,
|true logit| <~ 0.5 at these weight scales) and exp needs no row max.

keys are cast f32->f16 in-flight by the SWDGE DMA (gpsimd), so no
engine spends element time on the conversion.

Sharding: pure data parallel over batch, 256 rows/core on 8 cores.
"""

import sys

if "/opt/trn_rl_repo" not in sys.path:
    sys.path.insert(0, "/opt/trn_rl_repo")

import numpy as np

import concourse.bacc as bacc
import concourse.bass as bass
import concourse.tile as tile
from concourse import mybir
from concourse.bass_utils import run_bass_kernel_spmd

B, S, D, H, T, C = 2048, 512, 128, 4, 8, 64
Q = T * H  # 32
N_CORES = 8
ROWS = B // N_CORES  # 256 rows per core
GRP = 4  # batch rows per group -> 4*32 = 128 partitions
BLK = 128  # rows per block (ctx/QKT staging)
SC = 64.0  # power-of-two prescale keeping fp16 operands in normal range
MASK_NEG = 16384.0  # fp16/f32-exact; /SC = 256 pushes masked logits below -126

F32 = mybir.dt.float32
F16 = mybir.dt.float16

N_GRP = ROWS // GRP  # 64


def _emit(nc, tc, rows):
    """Emit the per-core program for `rows` batch rows (rows % GRP == 0)."""
    n_grp_total = rows // GRP
    keys_d = nc.declare_dram_parameter("keys", [rows, S, D], F32, isOutput=False)
    maskp_d = nc.declare_dram_parameter("maskp", [8, n_grp_total, S], F16,
                                        isOutput=False)
    ctx_d = nc.declare_dram_parameter("ctx", [rows, C], F32, isOutput=False)
    maug_d = nc.declare_dram_parameter("maug", [C + 1, Q, D], F16, isOutput=False)
    wvt_d = nc.declare_dram_parameter("wvt", [D, D], F16, isOutput=False)
    mstat_d = nc.declare_dram_parameter("mstat", [8, 128], F16, isOutput=False)
    id16_d = nc.declare_dram_parameter("id16", [128, 128], F16, isOutput=False)
    id32_d = nc.declare_dram_parameter("id32", [128, 128], F32, isOutput=False)
    out_d = nc.declare_dram_parameter("out", [rows, Q * D], F32, isOutput=True)

    keys_ap = keys_d.ap()
    ctx_ap = ctx_d.ap()
    out_ap = out_d.ap()

    n_blk = (rows + BLK - 1) // BLK

    import contextlib

    with contextlib.ExitStack() as ctx:
        singles = ctx.enter_context(tc.tile_pool(name="singles", bufs=1))
        kpool = ctx.enter_context(tc.tile_pool(name="kpool", bufs=8))
        ktpool = ctx.enter_context(tc.tile_pool(name="ktpool", bufs=4))
        blkpool = ctx.enter_context(tc.tile_pool(name="blkpool", bufs=2))
        qktpool = ctx.enter_context(tc.tile_pool(name="qktpool", bufs=2))
        work = ctx.enter_context(tc.tile_pool(name="work", bufs=4))
        small = ctx.enter_context(tc.tile_pool(name="small", bufs=6))
        ps = ctx.enter_context(tc.tile_pool(name="ps", bufs=1, space="PSUM"))

        # ---- constants (loaded once; ids first — they gate the first
        # transposes — big maskp last) ----
        id32_sb = singles.tile([128, 128], F32)
        nc.sync.dma_start(out=id32_sb, in_=id32_d.ap())
        id16_sb = singles.tile([128, 128], F16)
        nc.sync.dma_start(out=id16_sb, in_=id16_d.ap())
        maug_sb = singles.tile([C + 1, Q, D], F16)
        nc.sync.dma_start(out=maug_sb, in_=maug_d.ap())
        wvt_sb = singles.tile([D, D], F16)
        nc.sync.dma_start(out=wvt_sb, in_=wvt_d.ap())
        mstat_sb = singles.tile([8, 128], F16)
        nc.sync.dma_start(out=mstat_sb, in_=mstat_d.ap())
        maskp_sb = singles.tile([8, n_grp_total, S], F16)
        nc.sync.dma_start(out=maskp_sb, in_=maskp_d.ap())

        # Keys land s-permuted: partition p holds s = 4p+j (j = 0..3), so each
        # partition reads one contiguous 2 KiB run per row (line-rate DMA).
        # Pass 1/2 use the same s<->(p, j) mapping; maskp is host-permuted.
        staged = {}

        def _load_group(g):
            if g >= n_grp_total or g in staged:
                return
            g0 = g * GRP
            # SWDGE cast-DMA: HBM f32 -> SBUF fp16; one group per DMA so each
            # group's compute unblocks as soon as its own 1 MiB lands
            k16 = kpool.tile([128, GRP, S // 128, D], F16, tag="k16",
                             name=f"k16_{g}")
            nc.gpsimd.dma_start(
                out=k16,
                in_=keys_ap[g0 : g0 + GRP].rearrange("r (p j) d -> p r j d", p=128),
            )
            staged[g] = k16

        PF = 6  # software prefetch distance (groups)
        for g in range(PF):
            _load_group(g)

        # ---- prologue: conditioned queries qk'^T for every block ----
        qkt_blocks = []
        for blk in range(n_blk):
            r0 = blk * BLK
            bn = min(BLK, rows - r0)
            assert bn % GRP == 0

            ctx_sb = blkpool.tile([BLK, C], F32, tag="ctx")
            nc.sync.dma_start(out=ctx_sb[:bn], in_=ctx_ap[r0 : r0 + bn])
            ctxt_ps = ps.tile([C, BLK], F32, tag="t32", bufs=3)
            nc.tensor.transpose(ctxt_ps[:, :bn], ctx_sb[:bn], id32_sb[:bn, :bn])
            ctxt_sb = blkpool.tile([C + 1, BLK], F16, tag="ctxt")
            nc.vector.tensor_copy(out=ctxt_sb[:C, :bn], in_=ctxt_ps[:, :bn])
            nc.vector.memset(ctxt_sb[C : C + 1, :bn], 1.0)

            # qk'^T for the block: [D, bn, Q] fp16 (prescaled by SC*scale*inv_t)
            qkt_sb = qktpool.tile([D, BLK, Q], F16, tag="qkt")
            for q in range(Q):
                qkt_ps = ps.tile([D, BLK], F32, tag="t32", bufs=3)
                nc.tensor.matmul(
                    qkt_ps[:, :bn], maug_sb[:, q, :], ctxt_sb[:, :bn],
                    start=True, stop=True,
                )
                nc.vector.tensor_copy(out=qkt_sb[:, :bn, q], in_=qkt_ps[:, :bn])
            qkt_blocks.append(qkt_sb)

        for g in range(n_grp_total):
            g0 = g * GRP  # absolute row of this group
            qkt_sb = qkt_blocks[g0 // BLK]
            gl = (g0 % BLK) // GRP
            _load_group(g + PF)
            k16 = staged.pop(g)
            maskp_g = maskp_sb[:, g, :]

            # ---- keys^T per row: [d, s] via transpose-mode (fp16 PSUM out
            # halves the PSUM->SBUF copy cost; copies split Scalar/Vector) ----
            kt_sb = ktpool.tile([128, GRP, S], F16, tag="kt")
            for r in range(GRP):
                ktp = ps.tile([128, S], F16, tag="t16", bufs=3)
                for c in range(S // 128):
                    nc.tensor.transpose(
                        ktp[:, c * 128 : (c + 1) * 128], k16[:, r, c, :], id16_sb
                    )
                if r % 2 == 0:
                    nc.scalar.copy(out=kt_sb[:, r, :], in_=ktp)
                else:
                    nc.vector.tensor_copy(out=kt_sb[:, r, :], in_=ktp)

            # ---- pass 1: logits = qk'.keys + mask/alibi matmul ----
            lg_ps = ps.tile([128, S], F32, tag="logits", bufs=2)
            for r in range(GRP):
                nc.tensor.matmul(
                    lg_ps[32 * r : 32 * (r + 1), :],
                    qkt_sb[:, gl * GRP + r, :],
                    kt_sb[:, r, :],
                    start=True, stop=False,
                    tile_position=(0, 32 * r),
                    skip_group_check=True,
                )
            nc.tensor.matmul(
                lg_ps, mstat_sb, maskp_g,
                start=False, stop=True,
                skip_group_check=True,
            )

            # lg/SC is the exact shifted logit; exp directly from PSUM,
            # fp16 out, accumulator gives the softmax denominator.
            e_sb = work.tile([128, S], F16, tag="e")
            sum_sb = small.tile([128, 1], F32, tag="sum")
            nc.scalar.activation(
                out=e_sb, in_=lg_ps, func=mybir.ActivationFunctionType.Exp,
                scale=1.0 / SC, accum_out=sum_sb,
            )
            rs_sb = small.tile([128, 1], F32, tag="rs")
            nc.vector.reciprocal(rs_sb, sum_sb)

            # ---- w^T: [s_in_chunk, c, rq] fp16 via matmul-against-identity ----
            wt_ps = ps.tile([128, S // 128, 128], F32, tag="t32", bufs=3)
            for c in range(S // 128):
                nc.tensor.matmul(
                    wt_ps[:, c, :], e_sb[:, c * 128 : (c + 1) * 128],
                    id16_sb, start=True, stop=True,
                )
            wt_sb = work.tile([128, S // 128, 128], F16, tag="wt")
            nc.vector.tensor_copy(out=wt_sb, in_=wt_ps)

            # ---- pass 2: pk^T[d, rq] = sum_s keys[s,d] * w[rq,s] ----
            pk_ps = ps.tile([128, 128], F32, tag="t32", bufs=3)
            for r in range(GRP):
                for c in range(S // 128):
                    nc.tensor.matmul(
                        pk_ps[:, 32 * r : 32 * (r + 1)],
                        k16[:, r, c, :],
                        wt_sb[:, c, 32 * r : 32 * (r + 1)],
                        start=(c == 0), stop=(c == S // 128 - 1),
                        skip_group_check=True,
                    )
            pkt_sb = work.tile([128, 128], F16, tag="pkt")
            nc.vector.tensor_copy(out=pkt_sb, in_=pk_ps)

            # ---- pooled[rq, e] = pk^T.T @ Wv^T  (fp16 in, fp32 accum) ----
            po_ps = ps.tile([128, 128], F32, tag="t32", bufs=3)
            nc.tensor.matmul(po_ps, pkt_sb, wvt_sb, start=True, stop=True)

            o_sb = work.tile([128, 128], F32, tag="o")
            nc.vector.tensor_scalar(
                out=o_sb, in0=po_ps, scalar1=rs_sb, scalar2=None,
                op0=mybir.AluOpType.mult,
            )
            nc.sync.dma_start(
                out=out_ap[g0 : g0 + GRP].rearrange("r (q e) -> (r q) e", e=D),
                in_=o_sb,
            )


def _build(rows):
    nc = bacc.Bacc(trn_type="TRN2", target_bir_lowering=False, debug=False)
    with tile.TileContext(nc) as tc:
        _emit(nc, tc, rows)
    nc.compile()
    return nc


def host_consts(queries, Wk, log_temperature, Wc, bc, Wv):
    """Fold projections/scales into small host-side constants."""
    queries = np.asarray(queries, np.float64)
    Wk = np.asarray(Wk, np.float64)
    Wc = np.asarray(Wc, np.float64)
    bc = np.asarray(bc, np.float64)
    Wv = np.asarray(Wv, np.float64)
    lt = np.asarray(log_temperature, np.float64)

    scale = D ** -0.5
    inv_t = np.repeat(np.exp(-lt), H)  # [Q]
    slopes_h = 2.0 ** (-8.0 * (np.arange(H) + 1) / H)
    slopes = np.tile(slopes_h, T)  # [Q]
    s_q = scale * inv_t  # [Q]

    q_eff = queries.reshape(Q, D) + bc.reshape(Q, D)  # [Q, D]
    qk0 = q_eff @ Wk  # [Q, D]
    # maug[c, q, d]: rows 0..C-1 = SC*s_q * (Wc_q^T @ Wk); row C = SC*s_q * qk0
    maug = np.empty((C + 1, Q, D), np.float64)
    for q in range(Q):
        Wc_q = Wc[q * D : (q + 1) * D, :]  # [D(e), C]
        maug[:C, q, :] = (Wc_q.T @ Wk) * (SC * s_q[q])
        maug[C, q, :] = qk0[q] * (SC * s_q[q])

    # mstat: stationary for the mask/alibi matmul, k=8 partitions.
    # rows 0-3: select row r's 32 partitions (mask term)
    # rows 4-7: SC*slope[p] on row r's partitions (alibi term)
    slope_col = np.tile(slopes, 128 // Q)  # [128], p -> slopes[p % 32]
    mstat = np.zeros((8, 128), np.float16)
    for r in range(GRP):
        mstat[r, 32 * r : 32 * (r + 1)] = 1.0
        mstat[4 + r, 32 * r : 32 * (r + 1)] = SC * slope_col[32 * r : 32 * (r + 1)]

    return dict(
        maug=maug.astype(np.float16),
        wvt=np.ascontiguousarray(Wv.T).astype(np.float16),
        mstat=mstat,
        id16=np.eye(128, dtype=np.float16),
        id32=np.eye(128, dtype=np.float32),
    )


def make_in_maps(keys, mask, context, consts, rows, n_cores):
    keys = np.asarray(keys, np.float32)
    mask01 = np.asarray(mask, bool)
    ctx = np.asarray(context, np.float32)
    n_grp = rows // GRP
    s_ar = np.arange(S, dtype=np.float32)[None, None, :]  # [1,1,S]
    in_maps = []
    for i in range(n_cores):
        sl = slice(i * rows, (i + 1) * rows)
        m = mask01[sl].reshape(n_grp, GRP, S)  # [g, r, s]
        n_real = m.sum(axis=2, dtype=np.float32)  # [g, r]
        maskp = np.empty((8, n_grp, S), np.float32)
        # rows 0-3: 0 on real positions, -MASK_NEG on masked
        maskp[0:4] = (m.transpose(1, 0, 2).astype(np.float32) - 1.0) * MASK_NEG
        # rows 4-7: s - n_r + 1  (== -games_ago on real positions)
        maskp[4:8] = s_ar - n_real.T[:, :, None] + 1.0
        # device kt/lg column c' = j*128 + p holds s = 4p + j
        perm = 4 * (np.arange(S) % 128) + np.arange(S) // 128
        maskp = maskp[:, :, perm]
        in_maps.append(
            dict(
                keys=np.ascontiguousarray(keys[sl]),
                maskp=maskp.astype(np.float16),
                ctx=np.ascontiguousarray(ctx[sl]),
                **consts,
            )
        )
    return in_maps


_cache = {}


def run(keys, mask, context, queries, Wk, Wv, log_temperature, Wc, bc,
        trace=False, **kw):
    consts = host_consts(queries, Wk, log_temperature, Wc, bc, Wv)
    if ROWS not in _cache:
        _cache[ROWS] = _build(ROWS)
    nc = _cache[ROWS]
    in_maps = make_in_maps(keys, mask, context, consts, ROWS, N_CORES)
    res = run_bass_kernel_spmd(nc, in_maps, core_ids=list(range(N_CORES)),
                               trace=trace, **kw)
    out = np.concatenate([res.results[i]["out"] for i in range(N_CORES)], axis=0)
    return out.reshape(B, T, H * D).astype(np.float32), res


def kernel(keys, mask, context, queries, Wk, Wv, log_temperature, Wc, bc):
    out, _ = run(keys, mask, context, queries, Wk, Wv, log_temperature, Wc, bc)
    return out
